# revision 8
# baseline (speedup 1.0000x reference)
"""Block-diagonal linear for TRN2, 8 NeuronCores.

y = concat_h(x_h @ w_h + b_h) with x:[4,4096,4096] split into 16 blocks of
256 features; w:[16,256,256]; b:[16,256].

Sharding: data-parallel over rows. x is reshaped to [16384, 4096] and each
core takes 2048 contiguous rows; w and b are replicated. Zero communication.

Precision/bandwidth strategy: x and w are cast to bf16 on the host, y comes
back bf16 and is upcast on the host. This halves HBM traffic (the per-core
floor is ~358 GB/s) and runs the PE at 1 cycle/row. Accumulation stays fp32
in PSUM; bias is added in fp32 on DVE/Pool, rounding to bf16 once on output.

Per-core kernel (Tile framework):
  - w staged in SBUF as [128, 16, 2, 256] bf16 (contraction dim on partitions).
  - b broadcast across partitions once via gpsimd partition_broadcast (fp32).
  - For each 128-row tile of x: one DMA-transpose on the SP ring loads
    xT[p, c, r] = x[r, 128c+p] directly from HBM (bf16 is 2-byte, so the
    xbar transpose path applies) — no PE transposes, no PSUM staging, no
    ACT eviction copies.
  - 4 groups of 4 blocks: 8 bf16 matmuls per group accumulate fp32 into
    PSUM; DVE (groups 0,1,3) / Pool (group 2) add the fp32 bias while
    evicting PSUM->SBUF with a bf16 downcast.
  - y DMAs out bf16 on the ACT ring so loads and stores ride different
    HWDGE rings.
"""

import numpy as np
import ml_dtypes

import concourse.bacc as bacc
import concourse.mybir as mybir
from concourse import bass2jax, tile

N_CORES = 8
ROWS_TOTAL = 4 * 4096
ROWS = ROWS_TOTAL // N_CORES  # 2048 rows per core
WIDTH = 4096
NB = 16  # feature blocks
BW = 256  # block width
P = 128
M_TILES = ROWS // P  # 16

FP32 = mybir.dt.float32
BF16 = mybir.dt.bfloat16
NP_BF16 = ml_dtypes.bfloat16


def _build(repeat=1, xtb=3, yb=3, pyb=2):
    nc = bacc.Bacc(None, target_bir_lowering=False, debug=False)
    x = nc.dram_tensor("x", [ROWS, WIDTH], BF16, kind="ExternalInput")
    w = nc.dram_tensor("w", [NB, BW, BW], BF16, kind="ExternalInput")
    b = nc.dram_tensor("b", [NB, BW], FP32, kind="ExternalInput")
    y = nc.dram_tensor("y", [ROWS, WIDTH], BF16, kind="ExternalOutput")

    with tile.TileContext(nc) as tc:
        with (
            tc.tile_pool(name="const", bufs=1) as const_pool,
            tc.tile_pool(name="xtpool", bufs=xtb) as xt_pool,
            tc.tile_pool(name="ypool", bufs=yb) as y_pool,
            tc.tile_pool(name="py", bufs=pyb, space="PSUM") as psum_y,
        ):
            # Weights + bias on the ACT HWDGE ring so they don't head-block
            # the x-tile loads issued on the SP ring.
            # w_sb[p, h, ic, j] = w[h, ic*128+p, j]; staged in 4 chunks so the
            # first matmuls only wait for blocks 0-3.
            w_sb = const_pool.tile([P, NB, 2, BW], BF16)
            w_re = w.rearrange("h (ic p) j -> p h ic j", p=P)
            for q in range(4):
                nc.scalar.dma_start(
                    w_sb[:, 4 * q:4 * q + 4], w_re[:, 4 * q:4 * q + 4]
                )

            # Bias broadcast across partitions on GpSimd (Pool engine):
            # b_rep[p, h, j] = b[h, j].
            # b via the Pool SWDGE ring: overlaps the w staging on ACT at
            # startup, and Pool also does the broadcast right after.
            b_lin = const_pool.tile([1, NB, BW], FP32)
            nc.gpsimd.dma_start(
                b_lin[:], b.rearrange("(one h) j -> one h j", one=1)
            )
            b_rep = const_pool.tile([P, NB, BW], FP32)
            nc.gpsimd.partition_broadcast(
                b_rep[:].rearrange("p h j -> p (h j)"),
                b_lin[:].rearrange("o h j -> o (h j)"),
            )

            x_rows = x.rearrange("(t p) i -> t p i", p=P)
            y_rows = y.rearrange("(t p) i -> t p i", p=P)

            import contextlib

            rep_ctx = (
                tc.For_i(0, repeat, 1) if repeat > 1 else contextlib.nullcontext()
            )
            with rep_ctx:
                _main_loop(nc, tc, locals())

    nc.compile()
    return nc


def _main_loop(nc, tc, env):
    xt_pool = env["xt_pool"]
    y_pool = env["y_pool"]
    psum_y = env["psum_y"]
    w_sb = env["w_sb"]
    b_rep = env["b_rep"]
    x_rows = env["x_rows"]
    y_rows = env["y_rows"]
    for mi in range(M_TILES):
        # xT[p, c, r] = x[mi*128 + r, 128*c + p]: features onto partitions,
        # straight from HBM through the xbar transpose.
        xT = xt_pool.tile([P, WIDTH // P, P], BF16)
        nc.sync.dma_start_transpose(xT[:], x_rows[mi])

        y_t = y_pool.tile([P, NB, BW], BF16)
        for t in range(2):
            py = psum_y.tile([P, 8, BW], FP32)
            for u in range(8):
                h = 8 * t + u
                nc.tensor.matmul(
                    py[:, u, :], xT[:, 2 * h, :], w_sb[:, h, 0, :],
                    start=True, stop=False,
                )
                nc.tensor.matmul(
                    py[:, u, :], xT[:, 2 * h + 1, :], w_sb[:, h, 1, :],
                    start=False, stop=True,
                )
            # PSUM->SBUF eviction with fp32 bias add and bf16 downcast on
            # DVE (the only engine here that can both read PSUM and do
            # tensor+tensor; GPSIMD cannot access PSUM).
            nc.vector.tensor_add(
                y_t[:, 8 * t:8 * t + 8, :],
                py[:],
                b_rep[:, 8 * t:8 * t + 8, :],
            )
            if mi == M_TILES - 1:
                # Last tile: stream each half out right after its bias-add
                # so the kernel tail is just one small DMA.
                nc.scalar.dma_start(
                    y_rows[mi][:, t * 2048:(t + 1) * 2048],
                    y_t[:].rearrange("p h j -> p (h j)")[
                        :, t * 2048:(t + 1) * 2048
                    ],
                )
        if mi != M_TILES - 1:
            y_flat = y_t[:].rearrange("p h j -> p (h j)")
            hw = WIDTH // 2
            for q in range(2):
                nc.scalar.dma_start(
                    y_rows[mi][:, q * hw:(q + 1) * hw],
                    y_flat[:, q * hw:(q + 1) * hw],
                )


class _Runner:
    """Compile once, keep the jitted SPMD executable for reuse."""

    def __init__(self, repeat=1):
        import jax
        from jax.experimental.shard_map import shard_map
        from jax.sharding import Mesh, PartitionSpec

        self.jax = jax
        nc = _build(repeat=repeat)
        bass2jax.install_neuronx_cc_hook()

        assert nc.dbg_addr is None
        part_name = (
            nc.partition_id_tensor.name if nc.partition_id_tensor else None
        )
        in_names, out_names, out_avals = [], [], []
        for alloc in nc.m.functions[0].allocations:
            if not isinstance(alloc, mybir.MemoryLocationSet):
                continue
            name = alloc.memorylocations[0].name
            if alloc.kind == "ExternalInput":
                if name != part_name:
                    in_names.append(name)
            elif alloc.kind == "ExternalOutput":
                out_names.append(name)
                out_avals.append(
                    jax.core.ShapedArray(
                        tuple(alloc.tensor_shape), mybir.dt.np(alloc.dtype)
                    )
                )
        self.in_names = list(in_names)
        self.out_names = out_names
        self.out_avals = out_avals
        n_params = len(in_names)
        n_outs = len(out_names)
        all_names = in_names + out_names
        if part_name is not None:
            all_names = all_names + [part_name]

        def _body(*args):
            operands = list(args)
            if part_name is not None:
                operands.append(bass2jax.partition_id_tensor())
            outs = bass2jax._bass_exec_p.bind(
                *operands,
                out_avals=tuple(out_avals),
                in_names=tuple(all_names),
                out_names=tuple(out_names),
                lowering_input_output_aliases=(),
                sim_require_finite=True,
                sim_require_nnan=True,
                nc=nc,
            )
            return tuple(outs)

        devices = jax.devices()[:N_CORES]
        assert len(devices) == N_CORES
        self.mesh = Mesh(np.asarray(devices), ("core",))
        in_specs = (PartitionSpec("core"),) * (n_params + n_outs)
        out_specs = (PartitionSpec("core"),) * n_outs
        self.donate = tuple(range(n_params, n_params + n_outs))
        self.fn = jax.jit(
            shard_map(
                _body,
                mesh=self.mesh,
                in_specs=in_specs,
                out_specs=out_specs,
                check_rep=False,
            ),
            donate_argnums=self.donate,
            keep_unused=True,
        )

    def zeros(self):
        return [
            np.zeros((N_CORES * a.shape[0], *a.shape[1:]), a.dtype)
            for a in self.out_avals
        ]

    def prep(self, x, w, b):
        """Global (concatenated-over-cores) input arrays, in in_names order."""
        x2 = np.ascontiguousarray(
            np.asarray(x, dtype=np.float32).reshape(ROWS_TOTAL, WIDTH)
        ).astype(NP_BF16)
        w16 = np.asarray(w, dtype=np.float32).astype(NP_BF16)
        b32 = np.ascontiguousarray(np.asarray(b, dtype=np.float32))
        per = {
            "x": x2,
            "w": np.concatenate([w16] * N_CORES, axis=0),
            "b": np.concatenate([b32] * N_CORES, axis=0),
        }
        return [per[n] for n in self.in_names]

    def __call__(self, ins, zeros):
        outs = self.fn(*ins, *zeros)
        return dict(zip(self.out_names, outs))


_RUNNER = None


def _get_runner():
    global _RUNNER
    if _RUNNER is None:
        _RUNNER = _Runner()
    return _RUNNER


def kernel(x, w, b):
    r = _get_runner()
    outs = r(r.prep(x, w, b), r.zeros())
    y = np.asarray(outs["y"]).astype(np.float32)
    return y.reshape(4, 4096, WIDTH)


# revision 12
# speedup vs baseline: 1.3393x; 1.3393x over previous
"""Block-diagonal linear for TRN2, 8 NeuronCores.

y = concat_h(x_h @ w_h + b_h) with x:[4,4096,4096] split into 16 blocks of
256 features; w:[16,256,256]; b:[16,256].

Sharding: data-parallel over rows. x is reshaped to [16384, 4096] and each
core takes 2048 contiguous rows; w and b are replicated. Zero communication.

Precision/bandwidth strategy: x and w are cast to bf16 on the host, y comes
back bf16 and is upcast on the host. This halves HBM traffic and runs the PE
at 1 cycle/row. Accumulation stays fp32 in PSUM; bias is added in fp32 on
DVE, rounding to bf16 once on output.

Per-core kernel (Tile framework), engine budget per 128-row tile:
  - SP ring: x tile in, one 1MB DMA (~47us total).
  - PE: 32 bf16 transposes (features onto partitions, via identity matmul)
    + 32 bf16 matmuls, ~5.2us/tile (~84us total) -- the roofline engine.
  - ACT: evicts transpose PSUM->SBUF as bitcast uint32 (halves element
    count), plus w staging and half of each tile's y store.
  - DVE: fp32 bias-add + bf16 downcast eviction of the matmul PSUM,
    4 blocks at a time.
  - Pool (SWDGE): b staging/broadcast and the other half of y stores.
"""

import numpy as np
import ml_dtypes

import concourse.bacc as bacc
import concourse.mybir as mybir
from concourse import bass2jax, tile
from concourse.masks import make_identity

N_CORES = 8
ROWS_TOTAL = 4 * 4096
ROWS = ROWS_TOTAL // N_CORES  # 2048 rows per core
WIDTH = 4096
NB = 16  # feature blocks
BW = 256  # block width
P = 128
M_TILES = ROWS // P  # 16

FP32 = mybir.dt.float32
BF16 = mybir.dt.bfloat16
U32 = mybir.dt.uint32
NP_BF16 = ml_dtypes.bfloat16


def _build(repeat=1, xb=2, xtb=2, yb=2, ptb=2, pyb=3):
    nc = bacc.Bacc(None, target_bir_lowering=False, debug=False)
    x = nc.dram_tensor("x", [ROWS, WIDTH], BF16, kind="ExternalInput")
    w = nc.dram_tensor("w", [NB, BW, BW], BF16, kind="ExternalInput")
    b = nc.dram_tensor("b", [NB, BW], FP32, kind="ExternalInput")
    y = nc.dram_tensor("y", [ROWS, WIDTH], BF16, kind="ExternalOutput")

    with tile.TileContext(nc) as tc:
        with (
            tc.tile_pool(name="const", bufs=1) as const_pool,
            tc.tile_pool(name="xpool", bufs=xb) as x_pool,
            tc.tile_pool(name="xtpool", bufs=xtb) as xt_pool,
            tc.tile_pool(name="ypool", bufs=yb) as y_pool,
            tc.tile_pool(name="pt", bufs=ptb, space="PSUM") as psum_t,
            tc.tile_pool(name="py", bufs=pyb, space="PSUM") as psum_y,
        ):
            ident = const_pool.tile([P, P], BF16)
            make_identity(nc, ident[:])

            # w on the ACT HWDGE ring so it doesn't head-block the x-tile
            # loads issued on the SP ring. w_sb[p, h, ic, j] = w[h, ic*128+p, j]
            # staged in 4 chunks so the first matmuls only wait for blocks 0-3.
            w_sb = const_pool.tile([P, NB, 2, BW], BF16)
            w_re = w.rearrange("h (ic p) j -> p h ic j", p=P)
            for q in range(4):
                nc.scalar.dma_start(
                    w_sb[:, 4 * q:4 * q + 4], w_re[:, 4 * q:4 * q + 4]
                )

            # b via the Pool SWDGE ring + partition broadcast on Pool:
            # b_rep[p, h, j] = b[h, j]. Overlaps the w/x staging.
            b_lin = const_pool.tile([1, NB, BW], FP32)
            nc.gpsimd.dma_start(
                b_lin[:], b.rearrange("(one h) j -> one h j", one=1)
            )
            b_rep = const_pool.tile([P, NB, BW], FP32)
            nc.gpsimd.partition_broadcast(
                b_rep[:].rearrange("p h j -> p (h j)"),
                b_lin[:].rearrange("o h j -> o (h j)"),
            )

            x_rows = x.rearrange("(t p) i -> t p i", p=P)
            y_rows = y.rearrange("(t p) i -> t p i", p=P)

            import contextlib

            rep_ctx = (
                tc.For_i(0, repeat, 1) if repeat > 1 else contextlib.nullcontext()
            )
            with rep_ctx:
                _main_loop(nc, tc, locals())

    nc.compile()
    return nc


def _main_loop(nc, tc, env):
    x_pool = env["x_pool"]
    xt_pool = env["xt_pool"]
    y_pool = env["y_pool"]
    psum_t = env["psum_t"]
    psum_y = env["psum_y"]
    ident = env["ident"]
    w_sb = env["w_sb"]
    b_rep = env["b_rep"]
    x_rows = env["x_rows"]
    y_rows = env["y_rows"]
    for mi in range(M_TILES):
        x_t = x_pool.tile([P, WIDTH], BF16)
        hw = WIDTH // 2
        for q in range(2):
            nc.sync.dma_start(
                x_t[:, q * hw:(q + 1) * hw],
                x_rows[mi][:, q * hw:(q + 1) * hw],
            )

        # Per super-group t (4 blocks = 8 chunks): transpose the 8 chunks
        # into one full PSUM bank, evict to SBUF with one ACT copy as
        # uint32 (same bytes, half the elements), run the 8 accumulated
        # bf16 matmuls, then DVE adds the fp32 bias while evicting
        # PSUM->SBUF with a bf16 downcast.
        xT = xt_pool.tile([P, WIDTH // P, P], BF16)
        y_t = y_pool.tile([P, NB, BW], BF16)
        for t in range(4):
            pt = psum_t.tile([P, 8, P], BF16, tag="pt")
            for k in range(8):
                c = 8 * t + k
                nc.tensor.transpose(
                    pt[:, k, :], x_t[:, c * P:(c + 1) * P], ident[:]
                )
            nc.scalar.copy(xT[:, 8 * t:8 * t + 8, :], pt[:])

            py = psum_y.tile([P, 4, BW], FP32)
            for u in range(4):
                h = 4 * t + u
                nc.tensor.matmul(
                    py[:, u, :], xT[:, 2 * h, :], w_sb[:, h, 0, :],
                    start=True, stop=False,
                )
                nc.tensor.matmul(
                    py[:, u, :], xT[:, 2 * h + 1, :], w_sb[:, h, 1, :],
                    start=False, stop=True,
                )
            nc.vector.tensor_add(
                y_t[:, 4 * t:4 * t + 4, :],
                py[:],
                b_rep[:, 4 * t:4 * t + 4, :],
            )
            if mi == M_TILES - 1:
                # Last tile: stream each quarter out right after its
                # bias-add so the kernel tail is just one small DMA.
                eng = nc.scalar if t % 2 == 0 else nc.gpsimd
                eng.dma_start(
                    y_rows[mi][:, t * 1024:(t + 1) * 1024],
                    y_t[:].rearrange("p h j -> p (h j)")[
                        :, t * 1024:(t + 1) * 1024
                    ],
                )
            elif t == 1:
                # First half ready: store it now on the ACT ring while the
                # second half computes; second half goes out on Pool.
                nc.scalar.dma_start(
                    y_rows[mi][:, :hw],
                    y_t[:].rearrange("p h j -> p (h j)")[:, :hw],
                )
            elif t == 3:
                nc.gpsimd.dma_start(
                    y_rows[mi][:, hw:],
                    y_t[:].rearrange("p h j -> p (h j)")[:, hw:],
                )


class _Runner:
    """Compile once, keep the jitted SPMD executable for reuse."""

    def __init__(self, repeat=1):
        import jax
        from jax.experimental.shard_map import shard_map
        from jax.sharding import Mesh, PartitionSpec

        self.jax = jax
        nc = _build(repeat=repeat)
        bass2jax.install_neuronx_cc_hook()

        assert nc.dbg_addr is None
        part_name = (
            nc.partition_id_tensor.name if nc.partition_id_tensor else None
        )
        in_names, out_names, out_avals = [], [], []
        for alloc in nc.m.functions[0].allocations:
            if not isinstance(alloc, mybir.MemoryLocationSet):
                continue
            name = alloc.memorylocations[0].name
            if alloc.kind == "ExternalInput":
                if name != part_name:
                    in_names.append(name)
            elif alloc.kind == "ExternalOutput":
                out_names.append(name)
                out_avals.append(
                    jax.core.ShapedArray(
                        tuple(alloc.tensor_shape), mybir.dt.np(alloc.dtype)
                    )
                )
        self.in_names = list(in_names)
        self.out_names = out_names
        self.out_avals = out_avals
        n_params = len(in_names)
        n_outs = len(out_names)
        all_names = in_names + out_names
        if part_name is not None:
            all_names = all_names + [part_name]

        def _body(*args):
            operands = list(args)
            if part_name is not None:
                operands.append(bass2jax.partition_id_tensor())
            outs = bass2jax._bass_exec_p.bind(
                *operands,
                out_avals=tuple(out_avals),
                in_names=tuple(all_names),
                out_names=tuple(out_names),
                lowering_input_output_aliases=(),
                sim_require_finite=True,
                sim_require_nnan=True,
                nc=nc,
            )
            return tuple(outs)

        devices = jax.devices()[:N_CORES]
        assert len(devices) == N_CORES
        self.mesh = Mesh(np.asarray(devices), ("core",))
        in_specs = (PartitionSpec("core"),) * (n_params + n_outs)
        out_specs = (PartitionSpec("core"),) * n_outs
        self.donate = tuple(range(n_params, n_params + n_outs))
        self.fn = jax.jit(
            shard_map(
                _body,
                mesh=self.mesh,
                in_specs=in_specs,
                out_specs=out_specs,
                check_rep=False,
            ),
            donate_argnums=self.donate,
            keep_unused=True,
        )

    def zeros(self):
        return [
            np.zeros((N_CORES * a.shape[0], *a.shape[1:]), a.dtype)
            for a in self.out_avals
        ]

    def prep(self, x, w, b):
        """Global (concatenated-over-cores) input arrays, in in_names order."""
        x2 = np.ascontiguousarray(
            np.asarray(x, dtype=np.float32).reshape(ROWS_TOTAL, WIDTH)
        ).astype(NP_BF16)
        w16 = np.asarray(w, dtype=np.float32).astype(NP_BF16)
        b32 = np.ascontiguousarray(np.asarray(b, dtype=np.float32))
        per = {
            "x": x2,
            "w": np.concatenate([w16] * N_CORES, axis=0),
            "b": np.concatenate([b32] * N_CORES, axis=0),
        }
        return [per[n] for n in self.in_names]

    def __call__(self, ins, zeros):
        outs = self.fn(*ins, *zeros)
        return dict(zip(self.out_names, outs))


_RUNNER = None


def _get_runner():
    global _RUNNER
    if _RUNNER is None:
        _RUNNER = _Runner()
    return _RUNNER


def kernel(x, w, b):
    r = _get_runner()
    outs = r(r.prep(x, w, b), r.zeros())
    y = np.asarray(outs["y"]).astype(np.float32)
    return y.reshape(4, 4096, WIDTH)


# revision 27
# speedup vs baseline: 1.5185x; 1.1338x over previous
"""Block-diagonal linear for TRN2, 8 NeuronCores.

y = concat_h(x_h @ w_h + b_h) with x:[4,4096,4096] split into 16 blocks of
256 features; w:[16,256,256]; b:[16,256].

Sharding: data-parallel over rows. x is reshaped to [16384, 4096] and each
core takes 2048 contiguous rows; w and b are replicated. Zero communication.

Precision/bandwidth strategy: x and w are cast to bf16 on the host, y comes
back bf16 and is upcast on the host. This halves HBM traffic and runs the PE
at 1 cycle/row. Accumulation stays fp32 in PSUM; bias is added in fp32 on
DVE, rounding to bf16 once on output.

Per-core kernel (Tile framework), engine budget per 128-row tile:
  - SP ring: x tile in, 2x512KB DMAs.
  - PE: 32 bf16 transposes (features onto partitions, via identity matmul)
    + 32 bf16 matmuls at 1 cycle/row.
  - ACT: evicts the transpose PSUM->SBUF (one big copy per half-tile),
    plus w staging.
  - DVE: fp32 bias-add + bf16 downcast eviction of the matmul PSUM,
    4 blocks at a time.
  - Pool (SWDGE): b staging/broadcast and the y stores.
PSUM: transpose stage [128,16,128]bf16 = 2 banks x2 bufs, matmul
accumulators [128,4,256]fp32 = 2 banks x2 bufs (8 banks total; the 32-chunk
transpose lookahead is what keeps PE from stalling on evictions).

The PE stream is software-pipelined across half-tiles: back-to-back
transposes are LDWEIGHTS-bound (~107ns load vs 53ns stream) while matmuls
are stream-bound (~107ns stream, LDW hidden), so interleaving half H+1's
transposes 1:1 with half H's matmuls hides each op's weight load under the
other's moving stream (measured -19%: 120.5us -> 97.4us).

Measured via the hardware-loop delta method: ~97.4us/execution per core
(fp32 baseline: ~326us), rel err 3.7e-3 vs the 2e-2 gate.
"""

import numpy as np
import ml_dtypes

import concourse.bacc as bacc
import concourse.mybir as mybir
from concourse import bass2jax, tile
from concourse.masks import make_identity

N_CORES = 8
ROWS_TOTAL = 4 * 4096
ROWS = ROWS_TOTAL // N_CORES  # 2048 rows per core
WIDTH = 4096
NB = 16  # feature blocks
BW = 256  # block width
P = 128
M_TILES = ROWS // P  # 16

FP32 = mybir.dt.float32
BF16 = mybir.dt.bfloat16
NP_BF16 = ml_dtypes.bfloat16


def _build(repeat=1, unroll=1, xb=4, xtb=4, yb=4, ptb=2, pyb=2):
    nc = bacc.Bacc(None, target_bir_lowering=False, debug=False)
    x = nc.dram_tensor("x", [ROWS, WIDTH], BF16, kind="ExternalInput")
    w = nc.dram_tensor("w", [NB, BW, BW], BF16, kind="ExternalInput")
    b = nc.dram_tensor("b", [NB, BW], FP32, kind="ExternalInput")
    y = nc.dram_tensor("y", [ROWS, WIDTH], BF16, kind="ExternalOutput")

    with tile.TileContext(nc) as tc:
        with (
            tc.tile_pool(name="const", bufs=1) as const_pool,
            tc.tile_pool(name="xpool", bufs=xb) as x_pool,
            tc.tile_pool(name="xtpool", bufs=xtb) as xt_pool,
            tc.tile_pool(name="ypool", bufs=yb) as y_pool,
            tc.tile_pool(name="pt", bufs=ptb, space="PSUM") as psum_t,
            tc.tile_pool(name="py", bufs=pyb, space="PSUM") as psum_y,
        ):
            ident = const_pool.tile([P, P], BF16)
            make_identity(nc, ident[:])

            # w on the ACT HWDGE ring so it doesn't head-block the x-tile
            # loads issued on the SP ring. w_sb[p, h, ic, j] = w[h, ic*128+p, j]
            # staged in 4 chunks so the first matmuls only wait for blocks 0-3.
            w_sb = const_pool.tile([P, NB, 2, BW], BF16)
            w_re = w.rearrange("h (ic p) j -> p h ic j", p=P)
            for q in range(4):
                nc.scalar.dma_start(
                    w_sb[:, 4 * q:4 * q + 4], w_re[:, 4 * q:4 * q + 4]
                )

            # b via the Pool SWDGE ring + partition broadcast on Pool:
            # b_rep[p, h, j] = b[h, j]. Overlaps the w/x staging.
            b_lin = const_pool.tile([1, NB, BW], FP32)
            nc.gpsimd.dma_start(
                b_lin[:], b.rearrange("(one h) j -> one h j", one=1)
            )
            b_rep = const_pool.tile([P, NB, BW], FP32)
            nc.gpsimd.partition_broadcast(
                b_rep[:].rearrange("p h j -> p (h j)"),
                b_lin[:].rearrange("o h j -> o (h j)"),
            )

            x_rows = x.rearrange("(t p) i -> t p i", p=P)
            y_rows = y.rearrange("(t p) i -> t p i", p=P)

            import contextlib

            rep_ctx = (
                tc.For_i(0, repeat, 1) if repeat > 1 else contextlib.nullcontext()
            )
            with rep_ctx:
                for _ in range(unroll):
                    _main_loop(nc, tc, locals())

    nc.compile()
    return nc


def _main_loop(nc, tc, env):
    x_pool = env["x_pool"]
    xt_pool = env["xt_pool"]
    y_pool = env["y_pool"]
    psum_t = env["psum_t"]
    psum_y = env["psum_y"]
    ident = env["ident"]
    w_sb = env["w_sb"]
    b_rep = env["b_rep"]
    x_rows = env["x_rows"]
    y_rows = env["y_rows"]
    hw = WIDTH // 2
    NH = 2 * M_TILES  # half-tiles: 8 blocks = 16 chunks each

    # Software-pipelined over half-tiles: the PE instruction stream
    # alternates half H's matmuls (stream-bound, LDW hidden) with half
    # H+1's transposes (LDW-bound, stream short) so each op's weight
    # load hides under the other's moving stream.  The Tile framework
    # enforces the data deps; this only sets per-engine program order.
    tiles = {}

    def ensure_tile(mi):
        if mi in tiles:
            return tiles[mi]
        x_t = x_pool.tile([P, WIDTH], BF16)
        for q in range(2):
            nc.sync.dma_start(
                x_t[:, q * hw:(q + 1) * hw],
                x_rows[mi][:, q * hw:(q + 1) * hw],
            )
        xT = xt_pool.tile([P, WIDTH // P, P], BF16)
        y_t = y_pool.tile([P, NB, BW], BF16)
        tiles[mi] = (x_t, xT, y_t)
        return tiles[mi]

    def transpose_into(pt, H, k):
        x_t = ensure_tile(H // 2)[0]
        c = 16 * (H % 2) + k
        nc.tensor.transpose(pt[:, k, :], x_t[:, c * P:(c + 1) * P], ident[:])

    pt_cur = psum_t.tile([P, 16, P], BF16, tag="pt")
    for k in range(16):
        transpose_into(pt_cur, 0, k)

    for H in range(NH):
        mi, s = divmod(H, 2)
        _, xT, y_t = ensure_tile(mi)
        # Evict half H's transposes (one big ACT copy), freeing pt for H+2.
        nc.scalar.copy(xT[:, 16 * s:16 * s + 16, :], pt_cur[:])

        pt_next = None
        if H + 1 < NH:
            pt_next = psum_t.tile([P, 16, P], BF16, tag="pt")

        # 16 matmuls of half H interleaved 1:1 with 16 transposes of H+1.
        mm = [(u, ic) for u in range(8) for ic in range(2)]
        py = None
        for k in range(16):
            u, ic = mm[k]
            if ic == 0 and u % 4 == 0:
                py = psum_y.tile([P, 4, BW], FP32)
            h = 8 * s + u
            nc.tensor.matmul(
                py[:, u % 4, :], xT[:, 2 * h + ic, :], w_sb[:, h, ic, :],
                start=(ic == 0), stop=(ic == 1),
            )
            if pt_next is not None:
                transpose_into(pt_next, H + 1, k)
            if ic == 1 and u % 4 == 3:
                # Four blocks accumulated: DVE evicts with fp32 bias add
                # and bf16 downcast.
                t = 2 * s + u // 4
                nc.vector.tensor_add(
                    y_t[:, 4 * t:4 * t + 4, :],
                    py[:],
                    b_rep[:, 4 * t:4 * t + 4, :],
                )
        pt_cur = pt_next

        # Half H's 2048 output columns are final: stream them out. Split
        # across the SP HWDGE ring (which has headroom beside the x loads)
        # and the Pool SWDGE ring so neither DGE's per-DMA overhead stacks.
        eng = nc.sync if s == 0 else nc.gpsimd
        eng.dma_start(
            y_rows[mi][:, s * hw:(s + 1) * hw],
            y_t[:].rearrange("p h j -> p (h j)")[:, s * hw:(s + 1) * hw],
        )


class _Runner:
    """Compile once, keep the jitted SPMD executable for reuse."""

    def __init__(self, repeat=1, unroll=1):
        import jax
        from jax.experimental.shard_map import shard_map
        from jax.sharding import Mesh, PartitionSpec

        self.jax = jax
        nc = _build(repeat=repeat, unroll=unroll)
        bass2jax.install_neuronx_cc_hook()

        assert nc.dbg_addr is None
        part_name = (
            nc.partition_id_tensor.name if nc.partition_id_tensor else None
        )
        in_names, out_names, out_avals = [], [], []
        for alloc in nc.m.functions[0].allocations:
            if not isinstance(alloc, mybir.MemoryLocationSet):
                continue
            name = alloc.memorylocations[0].name
            if alloc.kind == "ExternalInput":
                if name != part_name:
                    in_names.append(name)
            elif alloc.kind == "ExternalOutput":
                out_names.append(name)
                out_avals.append(
                    jax.core.ShapedArray(
                        tuple(alloc.tensor_shape), mybir.dt.np(alloc.dtype)
                    )
                )
        self.in_names = list(in_names)
        self.out_names = out_names
        self.out_avals = out_avals
        n_params = len(in_names)
        n_outs = len(out_names)
        all_names = in_names + out_names
        if part_name is not None:
            all_names = all_names + [part_name]

        def _body(*args):
            operands = list(args)
            if part_name is not None:
                operands.append(bass2jax.partition_id_tensor())
            outs = bass2jax._bass_exec_p.bind(
                *operands,
                out_avals=tuple(out_avals),
                in_names=tuple(all_names),
                out_names=tuple(out_names),
                lowering_input_output_aliases=(),
                sim_require_finite=True,
                sim_require_nnan=True,
                nc=nc,
            )
            return tuple(outs)

        devices = jax.devices()[:N_CORES]
        assert len(devices) == N_CORES
        self.mesh = Mesh(np.asarray(devices), ("core",))
        in_specs = (PartitionSpec("core"),) * (n_params + n_outs)
        out_specs = (PartitionSpec("core"),) * n_outs
        self.donate = tuple(range(n_params, n_params + n_outs))
        self.fn = jax.jit(
            shard_map(
                _body,
                mesh=self.mesh,
                in_specs=in_specs,
                out_specs=out_specs,
                check_rep=False,
            ),
            donate_argnums=self.donate,
            keep_unused=True,
        )

    def zeros(self):
        return [
            np.zeros((N_CORES * a.shape[0], *a.shape[1:]), a.dtype)
            for a in self.out_avals
        ]

    def prep(self, x, w, b):
        """Global (concatenated-over-cores) input arrays, in in_names order."""
        x2 = np.ascontiguousarray(
            np.asarray(x, dtype=np.float32).reshape(ROWS_TOTAL, WIDTH)
        ).astype(NP_BF16)
        w16 = np.asarray(w, dtype=np.float32).astype(NP_BF16)
        b32 = np.ascontiguousarray(np.asarray(b, dtype=np.float32))
        per = {
            "x": x2,
            "w": np.concatenate([w16] * N_CORES, axis=0),
            "b": np.concatenate([b32] * N_CORES, axis=0),
        }
        return [per[n] for n in self.in_names]

    def __call__(self, ins, zeros):
        outs = self.fn(*ins, *zeros)
        return dict(zip(self.out_names, outs))


_RUNNER = None


def _get_runner():
    global _RUNNER
    if _RUNNER is None:
        _RUNNER = _Runner()
    return _RUNNER


def kernel(x, w, b):
    r = _get_runner()
    outs = r(r.prep(x, w, b), r.zeros())
    y = np.asarray(outs["y"]).astype(np.float32)
    return y.reshape(4, 4096, WIDTH)


# revision 31
# speedup vs baseline: 1.5308x; 1.0081x over previous
"""Block-diagonal linear for TRN2, 8 NeuronCores.

y = concat_h(x_h @ w_h + b_h) with x:[4,4096,4096] split into 16 blocks of
256 features; w:[16,256,256]; b:[16,256].

Sharding: data-parallel over rows. x is reshaped to [16384, 4096] and each
core takes 2048 contiguous rows; w and b are replicated. Zero communication.

Precision/bandwidth strategy: x and w are cast to bf16 on the host, y comes
back bf16 and is upcast on the host. This halves HBM traffic and runs the PE
at 1 cycle/row. Accumulation stays fp32 in PSUM; bias is added in fp32 on
DVE, rounding to bf16 once on output.

Per-core kernel (Tile framework), engine budget per 128-row tile:
  - SP ring: x tile in, 2x512KB DMAs.
  - PE: 32 bf16 transposes (features onto partitions, via identity matmul)
    + 32 bf16 matmuls at 1 cycle/row.
  - ACT: evicts the transpose PSUM->SBUF (one big copy per half-tile),
    plus w staging.
  - DVE: fp32 bias-add + bf16 downcast eviction of the matmul PSUM,
    4 blocks at a time.
  - Pool (SWDGE): b staging/broadcast and the y stores.
PSUM: transpose stage [128,16,128]bf16 = 2 banks x2 bufs, matmul
accumulators [128,4,256]fp32 = 2 banks x2 bufs (8 banks total; the 32-chunk
transpose lookahead is what keeps PE from stalling on evictions).

The PE stream is software-pipelined across half-tiles: back-to-back
transposes are LDWEIGHTS-bound (~107ns load vs 53ns stream) while matmuls
are stream-bound (~107ns stream, LDW hidden), so interleaving half H+1's
transposes 1:1 with half H's matmuls hides each op's weight load under the
other's moving stream (measured -19%: 120.5us -> 97.4us).

Measured via the hardware-loop delta method: ~97.4us/execution per core
(fp32 baseline: ~326us), rel err 3.7e-3 vs the 2e-2 gate.
"""

import numpy as np
import ml_dtypes

import concourse.bacc as bacc
import concourse.mybir as mybir
from concourse import bass2jax, tile
from concourse.masks import make_identity

N_CORES = 8
ROWS_TOTAL = 4 * 4096
ROWS = ROWS_TOTAL // N_CORES  # 2048 rows per core
WIDTH = 4096
NB = 16  # feature blocks
BW = 256  # block width
P = 128
M_TILES = ROWS // P  # 16

FP32 = mybir.dt.float32
BF16 = mybir.dt.bfloat16
NP_BF16 = ml_dtypes.bfloat16


def _build(repeat=1, unroll=1, xb=3, xtb=3, yb=3, ptb=4, pyb=2):
    nc = bacc.Bacc(None, target_bir_lowering=False, debug=False)
    x = nc.dram_tensor("x", [ROWS, WIDTH], BF16, kind="ExternalInput")
    w = nc.dram_tensor("w", [NB, BW, BW], BF16, kind="ExternalInput")
    b = nc.dram_tensor("b", [NB, BW], FP32, kind="ExternalInput")
    y = nc.dram_tensor("y", [ROWS, WIDTH], BF16, kind="ExternalOutput")

    with tile.TileContext(nc) as tc:
        with (
            tc.tile_pool(name="const", bufs=1) as const_pool,
            tc.tile_pool(name="xpool", bufs=xb) as x_pool,
            tc.tile_pool(name="xtpool", bufs=xtb) as xt_pool,
            tc.tile_pool(name="ypool", bufs=yb) as y_pool,
            tc.tile_pool(name="pt", bufs=ptb, space="PSUM") as psum_t,
            tc.tile_pool(name="py", bufs=pyb, space="PSUM") as psum_y,
        ):
            ident = const_pool.tile([P, P], BF16)
            make_identity(nc, ident[:])

            # w on the ACT HWDGE ring so it doesn't head-block the x-tile
            # loads issued on the SP ring. w_sb[p, h, ic, j] = w[h, ic*128+p, j]
            # staged in 4 chunks so the first matmuls only wait for blocks 0-3.
            w_sb = const_pool.tile([P, NB, 2, BW], BF16)
            w_re = w.rearrange("h (ic p) j -> p h ic j", p=P)
            for q in range(4):
                nc.scalar.dma_start(
                    w_sb[:, 4 * q:4 * q + 4], w_re[:, 4 * q:4 * q + 4]
                )

            # b via the Pool SWDGE ring + partition broadcast on Pool:
            # b_rep[p, h, j] = b[h, j]. Overlaps the w/x staging.
            b_lin = const_pool.tile([1, NB, BW], FP32)
            nc.gpsimd.dma_start(
                b_lin[:], b.rearrange("(one h) j -> one h j", one=1)
            )
            b_rep = const_pool.tile([P, NB, BW], FP32)
            nc.gpsimd.partition_broadcast(
                b_rep[:].rearrange("p h j -> p (h j)"),
                b_lin[:].rearrange("o h j -> o (h j)"),
            )

            x_rows = x.rearrange("(t p) i -> t p i", p=P)
            y_rows = y.rearrange("(t p) i -> t p i", p=P)

            import contextlib

            rep_ctx = (
                tc.For_i(0, repeat, 1) if repeat > 1 else contextlib.nullcontext()
            )
            with rep_ctx:
                for _ in range(unroll):
                    _main_loop(nc, tc, locals())

    nc.compile()
    return nc


def _main_loop(nc, tc, env):
    x_pool = env["x_pool"]
    xt_pool = env["xt_pool"]
    y_pool = env["y_pool"]
    psum_t = env["psum_t"]
    psum_y = env["psum_y"]
    ident = env["ident"]
    w_sb = env["w_sb"]
    b_rep = env["b_rep"]
    x_rows = env["x_rows"]
    y_rows = env["y_rows"]
    hw = WIDTH // 2
    NH = 2 * M_TILES  # half-tiles: 8 blocks = 16 chunks each

    # Software-pipelined over half-tiles: the PE instruction stream
    # alternates half H's matmuls (stream-bound, LDW hidden) with half
    # H+1's transposes (LDW-bound, stream short) so each op's weight
    # load hides under the other's moving stream.  The Tile framework
    # enforces the data deps; this only sets per-engine program order.
    tiles = {}

    def ensure_tile(mi):
        if mi in tiles:
            return tiles[mi]
        x_t = x_pool.tile([P, WIDTH], BF16)
        for q in range(2):
            nc.sync.dma_start(
                x_t[:, q * hw:(q + 1) * hw],
                x_rows[mi][:, q * hw:(q + 1) * hw],
            )
        xT = xt_pool.tile([P, WIDTH // P, P], BF16)
        y_t = y_pool.tile([P, NB, BW], BF16)
        tiles[mi] = (x_t, xT, y_t)
        return tiles[mi]

    def transpose_into(pt, H, k):
        x_t = ensure_tile(H // 2)[0]
        c = 16 * (H % 2) + k
        nc.tensor.transpose(
            pt[:, k % 8, :], x_t[:, c * P:(c + 1) * P], ident[:]
        )

    def evict(pt, H, lo):
        # One ACT copy of a fully-written 8-chunk pt tile (its own PSUM
        # bank — no partial reads of a tile PE is still writing).
        mi, s = divmod(H, 2)
        xT = ensure_tile(mi)[1]
        base = 16 * s + lo
        nc.scalar.copy(xT[:, base:base + 8, :], pt[:])

    # Prologue: transpose + evict half 0 in two 8-chunk stages, so half
    # 0's first matmuls only wait on the first (early) copy.
    for lo in (0, 8):
        pt = psum_t.tile([P, 8, P], BF16, tag="pt")
        for k in range(8):
            transpose_into(pt, 0, lo + k)
        evict(pt, 0, lo)

    for H in range(NH):
        mi, s = divmod(H, 2)
        _, xT, y_t = ensure_tile(mi)

        # 16 matmuls of half H interleaved 1:1 with 16 transposes of
        # H+1.  H+1's chunks land in two separate single-bank pt tiles,
        # each evicted as soon as it is complete — so when half H+1's
        # matmuls begin, their first 8 operand chunks are already in
        # SBUF and the second copy hides under the early matmuls.
        mm = [(u, ic) for u in range(8) for ic in range(2)]
        py = None
        pt = None
        for k in range(16):
            u, ic = mm[k]
            if ic == 0 and u % 4 == 0:
                py = psum_y.tile([P, 4, BW], FP32)
            h = 8 * s + u
            nc.tensor.matmul(
                py[:, u % 4, :], xT[:, 2 * h + ic, :], w_sb[:, h, ic, :],
                start=(ic == 0), stop=(ic == 1),
            )
            if H + 1 < NH:
                if k % 8 == 0:
                    pt = psum_t.tile([P, 8, P], BF16, tag="pt")
                transpose_into(pt, H + 1, k)
                if k % 8 == 7:
                    evict(pt, H + 1, k - 7)
            if ic == 1 and u % 4 == 3:
                # Four blocks accumulated: DVE evicts with fp32 bias add
                # and bf16 downcast.
                t = 2 * s + u // 4
                nc.vector.tensor_add(
                    y_t[:, 4 * t:4 * t + 4, :],
                    py[:],
                    b_rep[:, 4 * t:4 * t + 4, :],
                )

        # Half H's 2048 output columns are final: stream them out on the
        # Pool SWDGE ring, keeping ACT free for the PSUM evictions.
        nc.gpsimd.dma_start(
            y_rows[mi][:, s * hw:(s + 1) * hw],
            y_t[:].rearrange("p h j -> p (h j)")[:, s * hw:(s + 1) * hw],
        )


class _Runner:
    """Compile once, keep the jitted SPMD executable for reuse."""

    def __init__(self, repeat=1, unroll=1):
        import jax
        from jax.experimental.shard_map import shard_map
        from jax.sharding import Mesh, PartitionSpec

        self.jax = jax
        nc = _build(repeat=repeat, unroll=unroll)
        bass2jax.install_neuronx_cc_hook()

        assert nc.dbg_addr is None
        part_name = (
            nc.partition_id_tensor.name if nc.partition_id_tensor else None
        )
        in_names, out_names, out_avals = [], [], []
        for alloc in nc.m.functions[0].allocations:
            if not isinstance(alloc, mybir.MemoryLocationSet):
                continue
            name = alloc.memorylocations[0].name
            if alloc.kind == "ExternalInput":
                if name != part_name:
                    in_names.append(name)
            elif alloc.kind == "ExternalOutput":
                out_names.append(name)
                out_avals.append(
                    jax.core.ShapedArray(
                        tuple(alloc.tensor_shape), mybir.dt.np(alloc.dtype)
                    )
                )
        self.in_names = list(in_names)
        self.out_names = out_names
        self.out_avals = out_avals
        n_params = len(in_names)
        n_outs = len(out_names)
        all_names = in_names + out_names
        if part_name is not None:
            all_names = all_names + [part_name]

        def _body(*args):
            operands = list(args)
            if part_name is not None:
                operands.append(bass2jax.partition_id_tensor())
            outs = bass2jax._bass_exec_p.bind(
                *operands,
                out_avals=tuple(out_avals),
                in_names=tuple(all_names),
                out_names=tuple(out_names),
                lowering_input_output_aliases=(),
                sim_require_finite=True,
                sim_require_nnan=True,
                nc=nc,
            )
            return tuple(outs)

        devices = jax.devices()[:N_CORES]
        assert len(devices) == N_CORES
        self.mesh = Mesh(np.asarray(devices), ("core",))
        in_specs = (PartitionSpec("core"),) * (n_params + n_outs)
        out_specs = (PartitionSpec("core"),) * n_outs
        self.donate = tuple(range(n_params, n_params + n_outs))
        self.fn = jax.jit(
            shard_map(
                _body,
                mesh=self.mesh,
                in_specs=in_specs,
                out_specs=out_specs,
                check_rep=False,
            ),
            donate_argnums=self.donate,
            keep_unused=True,
        )

    def zeros(self):
        return [
            np.zeros((N_CORES * a.shape[0], *a.shape[1:]), a.dtype)
            for a in self.out_avals
        ]

    def prep(self, x, w, b):
        """Global (concatenated-over-cores) input arrays, in in_names order."""
        x2 = np.ascontiguousarray(
            np.asarray(x, dtype=np.float32).reshape(ROWS_TOTAL, WIDTH)
        ).astype(NP_BF16)
        w16 = np.asarray(w, dtype=np.float32).astype(NP_BF16)
        b32 = np.ascontiguousarray(np.asarray(b, dtype=np.float32))
        per = {
            "x": x2,
            "w": np.concatenate([w16] * N_CORES, axis=0),
            "b": np.concatenate([b32] * N_CORES, axis=0),
        }
        return [per[n] for n in self.in_names]

    def __call__(self, ins, zeros):
        outs = self.fn(*ins, *zeros)
        return dict(zip(self.out_names, outs))


_RUNNER = None


def _get_runner():
    global _RUNNER
    if _RUNNER is None:
        _RUNNER = _Runner()
    return _RUNNER


def kernel(x, w, b):
    r = _get_runner()
    outs = r(r.prep(x, w, b), r.zeros())
    y = np.asarray(outs["y"]).astype(np.float32)
    return y.reshape(4, 4096, WIDTH)
